# revision 16
# baseline (speedup 1.0000x reference)
"""LoRA layer kernel for Trainium2 (Bass/Tile), data-parallel over 8 NeuronCores.

Math:  out = (x @ B) @ A * (32/16)   with x [4,2048,4096], B [4096,16], A [16,4096].

Strategy (v3):
  - Flatten tokens (4*2048=8192), shard 1024 tokens per core (data parallel).
  - x pre-tiled on host as [ntb, 128, NB*tb] f16 so each partition's block is
    one contiguous DRAM run (8-16 KB descriptors, ~line-rate loads).
  - Single const DMA [128, NB*2R + OUT] f16: B as chunk-major lhsT tiles plus
    A pre-scaled and replicated into the 4 row groups (rows 32g+r = A[r]).
  - mm1: 4-way column-group packed f16 accumulation; col group g accumulates
    chunks {4k+g} into PSUM partitions [32g, 32g+32) (rows 32g+16.. are zero
    via B's zero padding).
  - mm2 contracts over all 128 partitions at once: lhsT = the f16 copy of the
    mm1 partials (stationary), rhs = replicated A. The 4-group reduction
    happens inside the matmul contraction -- no selector, no transposes.
  - mm2+evacuation of block k-1 is emitted interleaved with mm1 of block k so
    the PE never head-of-line blocks on PSUM bank recycling.
  - PSUM evacuation (f32 -> f16) split 10/6 between DVE and ACT.
"""

import os
import numpy as np

IN = 4096
OUT = 4096
R = 16
N_CORES = 8
SCALE = 32.0 / 16.0
P = 128
NB = IN // P  # 32 contraction chunks


def _install_profile_hook():
    """Best-effort: register the axon NTFF profiling hook that this image's
    `antenv` package is missing, so run_bass_kernel_spmd(trace=True) can
    return exec_time_ns. Harmless no-op when anything is unavailable."""
    try:
        import sys
        import types

        if "antenv.axon_hooks" in sys.modules:
            return
        try:
            import antenv  # noqa: F401
        except ImportError:
            return
        mod = types.ModuleType("antenv.axon_hooks")
        mod._hook = None

        def set_axon_ntff_profile_hook(h):
            mod._hook = h

        def get_axon_ntff_profile_hook():
            return mod._hook

        mod.set_axon_ntff_profile_hook = set_axon_ntff_profile_hook
        mod.get_axon_ntff_profile_hook = get_axon_ntff_profile_hook
        sys.modules["antenv.axon_hooks"] = mod
        import antenv as _antenv

        _antenv.axon_hooks = mod

        so_path = "/opt/axon/libaxon_pjrt.so"
        if os.path.exists(so_path):
            try:
                from trn_agent_boot.trn_boot import _ntff_profile_via_ctypes

                hook = _ntff_profile_via_ctypes(so_path)
                if hook is not None:
                    mod._hook = hook
            except Exception:
                pass
    except Exception:
        pass


_install_profile_hook()

_NC_CACHE = {}


def build_nc(tok, tb=256):
    """Build + compile the per-core Bass program for `tok` tokens/core."""
    key = (tok, tb)
    if key in _NC_CACHE:
        return _NC_CACHE[key]

    import concourse.bacc as bacc
    import concourse.tile as tile
    from concourse import mybir

    f32 = mybir.dt.float32
    f16 = mybir.dt.float16
    tb = min(tb, tok)
    assert tok % tb == 0 and tb % P == 0
    ntb = tok // tb
    nst = tb // P  # token subtiles per block
    fdim = NB * tb
    CB = NB * 2 * R  # const columns for B

    nc = bacc.Bacc("TRN2", target_bir_lowering=False, debug=False)
    xT = nc.dram_tensor("xT", [ntb, P, fdim], f16, kind="ExternalInput").ap()
    Bt = nc.dram_tensor("Bt", [P, NB, 2 * R], f16, kind="ExternalInput").ap()
    At = nc.dram_tensor("At", [2 * R, OUT], f16, kind="ExternalInput").ap()
    out = nc.dram_tensor("out", [tok, OUT], f16, kind="ExternalOutput").ap()

    with tile.TileContext(nc) as tc:
        with (
            tc.tile_pool(name="const", bufs=1) as const_pool,
            tc.tile_pool(name="xin", bufs=4) as x_pool,
            tc.tile_pool(name="part", bufs=2) as part_pool,
            tc.tile_pool(name="ps1", bufs=2, space="PSUM") as ps1,
            tc.tile_pool(name="ps2", bufs=3, space="PSUM") as ps2,
            tc.tile_pool(name="osb", bufs=2 + 2 * nst) as out_pool,
        ):
            # B ahead of the x blocks on the sync queue (mm1 needs it first);
            # A alone on the scalar queue, landing concurrently
            B_sb = const_pool.tile([P, NB, 2 * R], f16)
            nc.sync.dma_start(out=B_sb[:], in_=Bt[:])
            # A arrives as [A; zeros] (32 rows); two doubling copies build the
            # 4-row-group layout (rows 32g+r = A[r], rows 32g+16+.. = 0) with
            # no uninitialized SBUF in the matmul operand
            A_sb = const_pool.tile([P, OUT], f16)
            nc.scalar.dma_start(out=A_sb[0 : 2 * R, :], in_=At[:])
            for h in (32, 64):
                nc.vector.tensor_copy(A_sb[h : 2 * h, :], A_sb[0:h, :])

            def make_mm2(part_sb, o_sbs, tbi):
                """Emission closures for block tbi's mm2 + evacuation + stores."""
                emits = []
                for st in range(nst):
                    for op in range(OUT // 1024):

                        def emit(st=st, op=op, i=len(emits)):
                            ps_o = ps2.tile([P, 1024], f32)
                            for k in range(2):
                                nc.tensor.matmul(
                                    ps_o[:, k * 512 : (k + 1) * 512],
                                    lhsT=part_sb[:, st * P : (st + 1) * P],
                                    rhs=A_sb[:, (2 * op + k) * 512 : (2 * op + k + 1) * 512],
                                    start=True,
                                    stop=True,
                                    skip_group_check=True,
                                )
                            if i % 8 in (0, 2, 4, 6, 7):
                                nc.vector.tensor_copy(
                                    o_sbs[st][:, op * 1024 : (op + 1) * 1024], ps_o[:]
                                )
                            else:
                                nc.scalar.activation(
                                    o_sbs[st][:, op * 1024 : (op + 1) * 1024],
                                    ps_o[:],
                                    mybir.ActivationFunctionType.Copy,
                                )

                        emits.append(emit)

                def emit_stores():
                    for st in range(nst):
                        t0 = tbi * tb + st * P
                        nc.scalar.dma_start(out=out[t0 : t0 + P, :], in_=o_sbs[st][:])

                return emits, emit_stores

            pending, pending_stores = [], None
            for tbi in range(ntb):
                xT_sb = x_pool.tile([P, fdim], f16)
                nc.sync.dma_start(out=xT_sb[:], in_=xT[tbi])
                # drain half the previous block's mm2 before this block's mm1
                # (those only wait on PSUM banks, not on this block's x)
                nhead = len(pending) // 2
                for e in pending[:nhead]:
                    e()
                rest = pending[nhead:]
                # mm1: col group g accumulates chunks {4k+g} into partitions
                # [32g, 32g+32); rest of prev block's mm2 interleaves
                ps_part = ps1.tile([P, tb], f32)
                for c8 in range(NB // 4):
                    for g in range(4):
                        c = c8 * 4 + g
                        nc.tensor.matmul(
                            ps_part[32 * g : 32 * g + 2 * R, :],
                            lhsT=B_sb[:, c, :],
                            rhs=xT_sb[:, c * tb : (c + 1) * tb],
                            start=(c8 == 0),
                            stop=(c8 == NB // 4 - 1),
                            tile_position=(0, 32 * g),
                            skip_group_check=True,
                        )
                    for e in rest[c8 :: NB // 4]:
                        e()
                if pending_stores is not None:
                    pending_stores()
                part_sb = part_pool.tile([P, tb], f16, tag="part")
                nc.vector.tensor_copy(part_sb[:], ps_part[:])
                o_sbs = [
                    out_pool.tile([P, OUT], f16, name=f"osb{st}_{tbi}", tag=f"osb{st}")
                    for st in range(nst)
                ]
                pending, pending_stores = make_mm2(part_sb, o_sbs, tbi)
            for e in pending:
                e()
            pending_stores()

    nc.compile()
    _NC_CACHE[key] = nc
    return nc


TB = 256


def make_in_maps(x, lora_A, lora_B, n_cores=N_CORES):
    x = np.asarray(x, dtype=np.float32)
    A = np.asarray(lora_A, dtype=np.float32)
    B = np.asarray(lora_B, dtype=np.float32)
    xf = x.reshape(-1, IN)
    ntok = xf.shape[0] // n_cores
    tb = min(TB, ntok)
    # B chunk tiles, zero-padded to 32 wide (the zero columns make the mm1
    # partials zero in rows 32g+16.., which the mm2 contraction relies on)
    Bv = np.zeros((P, NB, 2 * R), dtype=np.float16)
    Bv[:, :, :R] = B.reshape(NB, P, R).transpose(1, 0, 2)
    A2 = np.zeros((2 * R, OUT), dtype=np.float16)
    A2[:R] = (A * np.float32(SCALE)).astype(np.float16)
    in_maps = []
    for c in range(n_cores):
        shard = xf[c * ntok : (c + 1) * ntok]
        # pre-tile: [ntb, 128, NB*tb]; xT[tbi,p,c*tb+t] = shard[tbi*tb+t, c*128+p]
        xt = np.ascontiguousarray(
            shard.reshape(ntok // tb, tb, NB, P)
            .transpose(0, 3, 2, 1)
            .reshape(ntok // tb, P, NB * tb),
            dtype=np.float16,
        )
        in_maps.append({"xT": xt, "Bt": Bv, "At": A2})
    return in_maps, ntok


def kernel_with_results(x, lora_A, lora_B, trace=False, **kwargs):
    from concourse.bass_utils import run_bass_kernel_spmd

    in_maps, ntok = make_in_maps(x, lora_A, lora_B)
    nc = build_nc(ntok, tb=TB)
    res = run_bass_kernel_spmd(nc, in_maps, list(range(N_CORES)), trace=trace, **kwargs)
    out = np.concatenate([r["out"] for r in res.results], axis=0).astype(np.float32)
    return out.reshape(np.asarray(x).shape[:-1] + (OUT,)), res


def kernel(x, lora_A, lora_B):
    out, _ = kernel_with_results(x, lora_A, lora_B)
    return out


# revision 19
# speedup vs baseline: 1.0074x; 1.0074x over previous
"""LoRA layer kernel for Trainium2 (Bass/Tile), data-parallel over 8 NeuronCores.

Math:  out = (x @ B) @ A * (32/16)   with x [4,2048,4096], B [4096,16], A [16,4096].

Strategy (v3):
  - Flatten tokens (4*2048=8192), shard 1024 tokens per core (data parallel).
  - x pre-tiled on host as [ntb, 128, NB*tb] f16 so each partition's block is
    one contiguous DRAM run (8-16 KB descriptors, ~line-rate loads).
  - Single const DMA [128, NB*2R + OUT] f16: B as chunk-major lhsT tiles plus
    A pre-scaled and replicated into the 4 row groups (rows 32g+r = A[r]).
  - mm1: 4-way column-group packed f16 accumulation; col group g accumulates
    chunks {4k+g} into PSUM partitions [32g, 32g+32) (rows 32g+16.. are zero
    via B's zero padding).
  - mm2 contracts over all 128 partitions at once: lhsT = the f16 copy of the
    mm1 partials (stationary), rhs = replicated A. The 4-group reduction
    happens inside the matmul contraction -- no selector, no transposes.
  - mm2+evacuation of block k-1 is emitted interleaved with mm1 of block k so
    the PE never head-of-line blocks on PSUM bank recycling.
  - PSUM evacuation (f32 -> f16) split 10/6 between DVE and ACT.
"""

import os
import numpy as np

IN = 4096
OUT = 4096
R = 16
N_CORES = 8
SCALE = 32.0 / 16.0
P = 128
NB = IN // P  # 32 contraction chunks


def _install_profile_hook():
    """Best-effort: register the axon NTFF profiling hook that this image's
    `antenv` package is missing, so run_bass_kernel_spmd(trace=True) can
    return exec_time_ns. Harmless no-op when anything is unavailable."""
    try:
        import sys
        import types

        if "antenv.axon_hooks" in sys.modules:
            return
        try:
            import antenv  # noqa: F401
        except ImportError:
            return
        mod = types.ModuleType("antenv.axon_hooks")
        mod._hook = None

        def set_axon_ntff_profile_hook(h):
            mod._hook = h

        def get_axon_ntff_profile_hook():
            return mod._hook

        mod.set_axon_ntff_profile_hook = set_axon_ntff_profile_hook
        mod.get_axon_ntff_profile_hook = get_axon_ntff_profile_hook
        sys.modules["antenv.axon_hooks"] = mod
        import antenv as _antenv

        _antenv.axon_hooks = mod

        so_path = "/opt/axon/libaxon_pjrt.so"
        if os.path.exists(so_path):
            try:
                from trn_agent_boot.trn_boot import _ntff_profile_via_ctypes

                hook = _ntff_profile_via_ctypes(so_path)
                if hook is not None:
                    mod._hook = hook
            except Exception:
                pass
    except Exception:
        pass


_install_profile_hook()

_NC_CACHE = {}


def build_nc(tok, tb=256):
    """Build + compile the per-core Bass program for `tok` tokens/core."""
    key = (tok, tb)
    if key in _NC_CACHE:
        return _NC_CACHE[key]

    import concourse.bacc as bacc
    import concourse.tile as tile
    from concourse import mybir

    f32 = mybir.dt.float32
    f16 = mybir.dt.float16
    tb = min(tb, tok)
    assert tok % tb == 0 and tb % P == 0
    ntb = tok // tb
    nst = tb // P  # token subtiles per block
    fdim = NB * tb
    CB = NB * 2 * R  # const columns for B

    nc = bacc.Bacc("TRN2", target_bir_lowering=False, debug=False)
    xT = nc.dram_tensor("xT", [ntb, P, fdim], f16, kind="ExternalInput").ap()
    Bt = nc.dram_tensor("Bt", [P, NB, 2 * R], f16, kind="ExternalInput").ap()
    At = nc.dram_tensor("At", [2 * R, OUT], f16, kind="ExternalInput").ap()
    out = nc.dram_tensor("out", [tok, OUT], f16, kind="ExternalOutput").ap()

    with tile.TileContext(nc) as tc:
        with (
            tc.tile_pool(name="const", bufs=1) as const_pool,
            tc.tile_pool(name="xin", bufs=4) as x_pool,
            tc.tile_pool(name="part", bufs=2) as part_pool,
            tc.tile_pool(name="ps1", bufs=2, space="PSUM") as ps1,
            tc.tile_pool(name="ps2", bufs=3, space="PSUM") as ps2,
            tc.tile_pool(name="osb", bufs=4 * nst) as out_pool,
        ):
            # B ahead of the x blocks on the sync queue (mm1 needs it first);
            # A alone on the scalar queue, landing concurrently
            B_sb = const_pool.tile([P, NB, 2 * R], f16)
            nc.sync.dma_start(out=B_sb[:], in_=Bt[:])
            # A arrives as [A; zeros] (32 rows); two doubling copies build the
            # 4-row-group layout (rows 32g+r = A[r], rows 32g+16+.. = 0) with
            # no uninitialized SBUF in the matmul operand
            A_sb = const_pool.tile([P, OUT], f16)
            nc.scalar.dma_start(out=A_sb[0 : 2 * R, :], in_=At[:])
            for h in (32, 64):
                nc.vector.tensor_copy(A_sb[h : 2 * h, :], A_sb[0:h, :])

            def make_mm2(part_sb, o_sbs, tbi):
                """Emission closures for block tbi's mm2 + evacuation + stores."""
                emits = []
                for st in range(nst):
                    for op in range(OUT // 1024):

                        def emit(st=st, op=op, i=len(emits)):
                            ps_o = ps2.tile([P, 1024], f32)
                            for k in range(2):
                                nc.tensor.matmul(
                                    ps_o[:, k * 512 : (k + 1) * 512],
                                    lhsT=part_sb[:, st * P : (st + 1) * P],
                                    rhs=A_sb[:, (2 * op + k) * 512 : (2 * op + k + 1) * 512],
                                    start=True,
                                    stop=True,
                                    skip_group_check=True,
                                )
                            if i % 2 == 0:
                                nc.vector.tensor_copy(
                                    o_sbs[st][:, op * 1024 : (op + 1) * 1024], ps_o[:]
                                )
                            else:
                                nc.scalar.activation(
                                    o_sbs[st][:, op * 1024 : (op + 1) * 1024],
                                    ps_o[:],
                                    mybir.ActivationFunctionType.Copy,
                                )

                        emits.append(emit)

                def emit_stores():
                    for st in range(nst):
                        t0 = tbi * tb + st * P
                        if tbi == ntb - 1 and st == nst - 1:
                            # last tile: taper the store so the final DMA piece
                            # (whose completion latency ends the kernel) is small
                            for c0, c1 in ((0, 2048), (2048, 3072), (3072, 4096)):
                                nc.sync.dma_start(
                                    out=out[t0 : t0 + P, c0:c1],
                                    in_=o_sbs[st][:, c0:c1],
                                )
                        else:
                            nc.sync.dma_start(out=out[t0 : t0 + P, :], in_=o_sbs[st][:])

                return emits, emit_stores

            pending, pending_stores = [], None
            for tbi in range(ntb):
                xT_sb = x_pool.tile([P, fdim], f16)
                nc.sync.dma_start(out=xT_sb[:], in_=xT[tbi])
                # drain half the previous block's mm2 before this block's mm1
                # (those only wait on PSUM banks, not on this block's x)
                nhead = len(pending) // 2
                for e in pending[:nhead]:
                    e()
                rest = pending[nhead:]
                # mm1: col group g accumulates chunks {4k+g} into partitions
                # [32g, 32g+32); rest of prev block's mm2 interleaves
                ps_part = ps1.tile([P, tb], f32)
                for c8 in range(NB // 4):
                    for g in range(4):
                        c = c8 * 4 + g
                        nc.tensor.matmul(
                            ps_part[32 * g : 32 * g + 2 * R, :],
                            lhsT=B_sb[:, c, :],
                            rhs=xT_sb[:, c * tb : (c + 1) * tb],
                            start=(c8 == 0),
                            stop=(c8 == NB // 4 - 1),
                            tile_position=(0, 32 * g),
                            skip_group_check=True,
                        )
                    for e in rest[c8 :: NB // 4]:
                        e()
                if pending_stores is not None:
                    pending_stores()
                part_sb = part_pool.tile([P, tb], f16, tag="part")
                nc.vector.tensor_copy(part_sb[:], ps_part[:])
                o_sbs = [
                    out_pool.tile([P, OUT], f16, name=f"osb{st}_{tbi}", tag=f"osb{st}")
                    for st in range(nst)
                ]
                pending, pending_stores = make_mm2(part_sb, o_sbs, tbi)
            for e in pending:
                e()
            pending_stores()

    nc.compile()
    _NC_CACHE[key] = nc
    return nc


TB = 256


def make_in_maps(x, lora_A, lora_B, n_cores=N_CORES):
    x = np.asarray(x, dtype=np.float32)
    A = np.asarray(lora_A, dtype=np.float32)
    B = np.asarray(lora_B, dtype=np.float32)
    xf = x.reshape(-1, IN)
    ntok = xf.shape[0] // n_cores
    tb = min(TB, ntok)
    # B chunk tiles, zero-padded to 32 wide (the zero columns make the mm1
    # partials zero in rows 32g+16.., which the mm2 contraction relies on)
    Bv = np.zeros((P, NB, 2 * R), dtype=np.float16)
    Bv[:, :, :R] = B.reshape(NB, P, R).transpose(1, 0, 2)
    A2 = np.zeros((2 * R, OUT), dtype=np.float16)
    A2[:R] = (A * np.float32(SCALE)).astype(np.float16)
    in_maps = []
    for c in range(n_cores):
        shard = xf[c * ntok : (c + 1) * ntok]
        # pre-tile: [ntb, 128, NB*tb]; xT[tbi,p,c*tb+t] = shard[tbi*tb+t, c*128+p]
        xt = np.ascontiguousarray(
            shard.reshape(ntok // tb, tb, NB, P)
            .transpose(0, 3, 2, 1)
            .reshape(ntok // tb, P, NB * tb),
            dtype=np.float16,
        )
        in_maps.append({"xT": xt, "Bt": Bv, "At": A2})
    return in_maps, ntok


def kernel_with_results(x, lora_A, lora_B, trace=False, **kwargs):
    from concourse.bass_utils import run_bass_kernel_spmd

    in_maps, ntok = make_in_maps(x, lora_A, lora_B)
    nc = build_nc(ntok, tb=TB)
    res = run_bass_kernel_spmd(nc, in_maps, list(range(N_CORES)), trace=trace, **kwargs)
    out = np.concatenate([r["out"] for r in res.results], axis=0).astype(np.float32)
    return out.reshape(np.asarray(x).shape[:-1] + (OUT,)), res


def kernel(x, lora_A, lora_B):
    out, _ = kernel_with_results(x, lora_A, lora_B)
    return out


# revision 21
# speedup vs baseline: 1.2610x; 1.2518x over previous
"""LoRA layer kernel for Trainium2 (Bass/Tile), data-parallel over 8 NeuronCores.

Math:  out = (x @ B) @ A * (32/16)   with x [4,2048,4096], B [4096,16], A [16,4096].

Strategy (v3):
  - Flatten tokens (4*2048=8192), shard 1024 tokens per core (data parallel).
  - x pre-tiled on host as [ntb, 128, NB*tb] f16 so each partition's block is
    one contiguous DRAM run (8-16 KB descriptors, ~line-rate loads).
  - Single const DMA [128, NB*2R + OUT] f16: B as chunk-major lhsT tiles plus
    A pre-scaled and replicated into the 4 row groups (rows 32g+r = A[r]).
  - mm1: 4-way column-group packed f16 accumulation; col group g accumulates
    chunks {4k+g} into PSUM partitions [32g, 32g+32) (rows 32g+16.. are zero
    via B's zero padding).
  - mm2 contracts over all 128 partitions at once: lhsT = the f16 copy of the
    mm1 partials (stationary), rhs = replicated A. The 4-group reduction
    happens inside the matmul contraction -- no selector, no transposes.
  - mm2+evacuation of block k-1 is emitted interleaved with mm1 of block k so
    the PE never head-of-line blocks on PSUM bank recycling.
  - PSUM evacuation (f32 -> f16) split 10/6 between DVE and ACT.
"""

import os
import numpy as np

IN = 4096
OUT = 4096
R = 16
N_CORES = 8
SCALE = 32.0 / 16.0
P = 128
NB = IN // P  # 32 contraction chunks


def _install_profile_hook():
    """Best-effort: register the axon NTFF profiling hook that this image's
    `antenv` package is missing, so run_bass_kernel_spmd(trace=True) can
    return exec_time_ns. Harmless no-op when anything is unavailable."""
    try:
        import sys
        import types

        if "antenv.axon_hooks" in sys.modules:
            return
        try:
            import antenv  # noqa: F401
        except ImportError:
            return
        mod = types.ModuleType("antenv.axon_hooks")
        mod._hook = None

        def set_axon_ntff_profile_hook(h):
            mod._hook = h

        def get_axon_ntff_profile_hook():
            return mod._hook

        mod.set_axon_ntff_profile_hook = set_axon_ntff_profile_hook
        mod.get_axon_ntff_profile_hook = get_axon_ntff_profile_hook
        sys.modules["antenv.axon_hooks"] = mod
        import antenv as _antenv

        _antenv.axon_hooks = mod

        so_path = "/opt/axon/libaxon_pjrt.so"
        if os.path.exists(so_path):
            try:
                from trn_agent_boot.trn_boot import _ntff_profile_via_ctypes

                hook = _ntff_profile_via_ctypes(so_path)
                if hook is not None:
                    mod._hook = hook
            except Exception:
                pass
    except Exception:
        pass


_install_profile_hook()

_NC_CACHE = {}


def build_nc(tok, tb=256):
    """Build + compile the per-core Bass program for `tok` tokens/core."""
    key = (tok, tb)
    if key in _NC_CACHE:
        return _NC_CACHE[key]

    import concourse.bacc as bacc
    import concourse.tile as tile
    from concourse import mybir

    f32 = mybir.dt.float32
    f16 = mybir.dt.float16
    tb = min(tb, tok)
    assert tok % tb == 0 and tb % P == 0
    ntb = tok // tb
    nst = tb // P  # token subtiles per block
    fdim = NB * tb
    CB = NB * 2 * R  # const columns for B

    nc = bacc.Bacc("TRN2", target_bir_lowering=False, debug=False)
    xT = nc.dram_tensor("xT", [ntb, P, fdim], f16, kind="ExternalInput").ap()
    Bt = nc.dram_tensor("Bt", [P, NB, 2 * R], f16, kind="ExternalInput").ap()
    At = nc.dram_tensor("At", [2 * R, OUT], f16, kind="ExternalInput").ap()
    out = nc.dram_tensor("out", [tok, OUT], f16, kind="ExternalOutput").ap()

    with tile.TileContext(nc) as tc:
        with (
            tc.tile_pool(name="const", bufs=1) as const_pool,
            tc.tile_pool(name="xin", bufs=ntb) as x_pool,
            tc.tile_pool(name="part", bufs=2) as part_pool,
            tc.tile_pool(name="ps1", bufs=2, space="PSUM") as ps1,
            tc.tile_pool(name="ps2", bufs=3, space="PSUM") as ps2,
            tc.tile_pool(name="osb", bufs=4 * nst) as out_pool,
        ):
            # B ahead of the x blocks on the sync queue (mm1 needs it first);
            # A alone on the scalar queue, landing concurrently
            B_sb = const_pool.tile([P, NB, 2 * R], f16)
            nc.sync.dma_start(out=B_sb[:], in_=Bt[:])
            # A arrives as [A; zeros] (32 rows); two doubling copies build the
            # 4-row-group layout (rows 32g+r = A[r], rows 32g+16+.. = 0) with
            # no uninitialized SBUF in the matmul operand
            A_sb = const_pool.tile([P, OUT], f16)
            nc.scalar.dma_start(out=A_sb[0 : 2 * R, :], in_=At[:])
            for h in (32, 64):
                nc.vector.tensor_copy(A_sb[h : 2 * h, :], A_sb[0:h, :])

            def make_mm2(part_sb, o_sbs, tbi):
                """Emission closures for block tbi's mm2 + evacuation + stores."""
                emits = []
                for st in range(nst):
                    for op in range(OUT // 1024):

                        def emit(st=st, op=op, i=len(emits)):
                            ps_o = ps2.tile([P, 1024], f32)
                            for k in range(2):
                                nc.tensor.matmul(
                                    ps_o[:, k * 512 : (k + 1) * 512],
                                    lhsT=part_sb[:, st * P : (st + 1) * P],
                                    rhs=A_sb[:, (2 * op + k) * 512 : (2 * op + k + 1) * 512],
                                    start=True,
                                    stop=True,
                                    skip_group_check=True,
                                )
                            if i % 2 == 0:
                                nc.vector.tensor_copy(
                                    o_sbs[st][:, op * 1024 : (op + 1) * 1024], ps_o[:]
                                )
                            else:
                                nc.scalar.activation(
                                    o_sbs[st][:, op * 1024 : (op + 1) * 1024],
                                    ps_o[:],
                                    mybir.ActivationFunctionType.Copy,
                                )

                        emits.append(emit)

                def emit_stores():
                    for st in range(nst):
                        t0 = tbi * tb + st * P
                        if tbi == ntb - 1 and st == nst - 1:
                            # last tile: taper the store so the final DMA piece
                            # (whose completion latency ends the kernel) is small
                            for c0, c1 in ((0, 2048), (2048, 3072), (3072, 4096)):
                                nc.sync.dma_start(
                                    out=out[t0 : t0 + P, c0:c1],
                                    in_=o_sbs[st][:, c0:c1],
                                )
                        else:
                            nc.sync.dma_start(out=out[t0 : t0 + P, :], in_=o_sbs[st][:])

                return emits, emit_stores

            # dispatch every x load up-front: a store dispatch on the sync
            # engine is sem-gated on evacuation, and any load emitted after it
            # would be head-of-line blocked behind that wait
            x_tiles = []
            for tbi in range(ntb):
                xT_sb = x_pool.tile([P, fdim], f16)
                nc.sync.dma_start(out=xT_sb[:], in_=xT[tbi])
                x_tiles.append(xT_sb)

            pending, pending_stores = [], None
            for tbi in range(ntb):
                xT_sb = x_tiles[tbi]
                # drain half the previous block's mm2 before this block's mm1
                # (those only wait on PSUM banks, not on this block's x)
                nhead = len(pending) // 2
                for e in pending[:nhead]:
                    e()
                rest = pending[nhead:]
                # mm1: col group g accumulates chunks {4k+g} into partitions
                # [32g, 32g+32); rest of prev block's mm2 interleaves
                ps_part = ps1.tile([P, tb], f32)
                for c8 in range(NB // 4):
                    for g in range(4):
                        c = c8 * 4 + g
                        nc.tensor.matmul(
                            ps_part[32 * g : 32 * g + 2 * R, :],
                            lhsT=B_sb[:, c, :],
                            rhs=xT_sb[:, c * tb : (c + 1) * tb],
                            start=(c8 == 0),
                            stop=(c8 == NB // 4 - 1),
                            tile_position=(0, 32 * g),
                            skip_group_check=True,
                        )
                    for e in rest[c8 :: NB // 4]:
                        e()
                if pending_stores is not None:
                    pending_stores()
                part_sb = part_pool.tile([P, tb], f16, tag="part")
                nc.vector.tensor_copy(part_sb[:], ps_part[:])
                o_sbs = [
                    out_pool.tile([P, OUT], f16, name=f"osb{st}_{tbi}", tag=f"osb{st}")
                    for st in range(nst)
                ]
                pending, pending_stores = make_mm2(part_sb, o_sbs, tbi)
            for e in pending:
                e()
            pending_stores()

    nc.compile()
    _NC_CACHE[key] = nc
    return nc


TB = 256


def make_in_maps(x, lora_A, lora_B, n_cores=N_CORES):
    x = np.asarray(x, dtype=np.float32)
    A = np.asarray(lora_A, dtype=np.float32)
    B = np.asarray(lora_B, dtype=np.float32)
    xf = x.reshape(-1, IN)
    ntok = xf.shape[0] // n_cores
    tb = min(TB, ntok)
    # B chunk tiles, zero-padded to 32 wide (the zero columns make the mm1
    # partials zero in rows 32g+16.., which the mm2 contraction relies on)
    Bv = np.zeros((P, NB, 2 * R), dtype=np.float16)
    Bv[:, :, :R] = B.reshape(NB, P, R).transpose(1, 0, 2)
    A2 = np.zeros((2 * R, OUT), dtype=np.float16)
    A2[:R] = (A * np.float32(SCALE)).astype(np.float16)
    in_maps = []
    for c in range(n_cores):
        shard = xf[c * ntok : (c + 1) * ntok]
        # pre-tile: [ntb, 128, NB*tb]; xT[tbi,p,c*tb+t] = shard[tbi*tb+t, c*128+p]
        xt = np.ascontiguousarray(
            shard.reshape(ntok // tb, tb, NB, P)
            .transpose(0, 3, 2, 1)
            .reshape(ntok // tb, P, NB * tb),
            dtype=np.float16,
        )
        in_maps.append({"xT": xt, "Bt": Bv, "At": A2})
    return in_maps, ntok


def kernel_with_results(x, lora_A, lora_B, trace=False, **kwargs):
    from concourse.bass_utils import run_bass_kernel_spmd

    in_maps, ntok = make_in_maps(x, lora_A, lora_B)
    nc = build_nc(ntok, tb=TB)
    res = run_bass_kernel_spmd(nc, in_maps, list(range(N_CORES)), trace=trace, **kwargs)
    out = np.concatenate([r["out"] for r in res.results], axis=0).astype(np.float32)
    return out.reshape(np.asarray(x).shape[:-1] + (OUT,)), res


def kernel(x, lora_A, lora_B):
    out, _ = kernel_with_results(x, lora_A, lora_B)
    return out


# revision 22
# speedup vs baseline: 1.2783x; 1.0137x over previous
"""LoRA layer kernel for Trainium2 (Bass/Tile), data-parallel over 8 NeuronCores.

Math:  out = (x @ B) @ A * (32/16)   with x [4,2048,4096], B [4096,16], A [16,4096].

Strategy:
  - Flatten tokens (4*2048=8192), shard 1024 tokens per core (data parallel).
  - x pre-tiled on host as [ntb, 128, NB*tb] f16 so each partition's block is
    one contiguous DRAM run (8-16 KB descriptors, ~line-rate loads). All x
    load dispatches are emitted up-front so no store dispatch (sem-gated on
    evacuation) can head-of-line block them on the sync engine.
  - Consts are tiny: B [128, NB, 32] f16 ahead of x on the sync queue, A
    [32, OUT] f16 on the scalar queue + two on-chip doubling copies to build
    the replicated-A layout (rows 32g+r = A_scaled[r]).
  - mm1: 4-way column-group packed f16 accumulation; col group g accumulates
    chunks {4k+g} into PSUM partitions [32g, 32g+32) (rows 32g+16.. are zero
    via B's zero padding).
  - mm2 contracts over all 128 partitions at once: lhsT = the f16 copy of the
    mm1 partials (stationary), rhs = replicated A. The 4-group reduction
    happens inside the matmul contraction -- no selector, no transposes.
  - mm2+evacuation of block k-1 is emitted interleaved with mm1 of block k so
    the PE never head-of-line blocks on PSUM bank recycling; PSUM evacuation
    (f32 -> f16, two banks per copy) alternates DVE/ACT.
  - The final store is tapered so the last DMA's completion latency covers
    only a 256 KB piece.

Measured on 8xTRN2: HW exec ~55 us (baseline 89.6, DMA roofline ~53): loads
+ stores sustain ~389 GB/s/core with compute fully hidden underneath.
"""

import os
import numpy as np

IN = 4096
OUT = 4096
R = 16
N_CORES = 8
SCALE = 32.0 / 16.0
P = 128
NB = IN // P  # 32 contraction chunks


def _install_profile_hook():
    """Best-effort: register the axon NTFF profiling hook that this image's
    `antenv` package is missing, so run_bass_kernel_spmd(trace=True) can
    return exec_time_ns. Harmless no-op when anything is unavailable."""
    try:
        import sys
        import types

        if "antenv.axon_hooks" in sys.modules:
            return
        try:
            import antenv  # noqa: F401
        except ImportError:
            return
        mod = types.ModuleType("antenv.axon_hooks")
        mod._hook = None

        def set_axon_ntff_profile_hook(h):
            mod._hook = h

        def get_axon_ntff_profile_hook():
            return mod._hook

        mod.set_axon_ntff_profile_hook = set_axon_ntff_profile_hook
        mod.get_axon_ntff_profile_hook = get_axon_ntff_profile_hook
        sys.modules["antenv.axon_hooks"] = mod
        import antenv as _antenv

        _antenv.axon_hooks = mod

        so_path = "/opt/axon/libaxon_pjrt.so"
        if os.path.exists(so_path):
            try:
                from trn_agent_boot.trn_boot import _ntff_profile_via_ctypes

                hook = _ntff_profile_via_ctypes(so_path)
                if hook is not None:
                    mod._hook = hook
            except Exception:
                pass
    except Exception:
        pass


_install_profile_hook()

_NC_CACHE = {}


def build_nc(tok, tb=256):
    """Build + compile the per-core Bass program for `tok` tokens/core."""
    key = (tok, tb)
    if key in _NC_CACHE:
        return _NC_CACHE[key]

    import concourse.bacc as bacc
    import concourse.tile as tile
    from concourse import mybir

    f32 = mybir.dt.float32
    f16 = mybir.dt.float16
    tb = min(tb, tok)
    assert tok % tb == 0 and tb % P == 0
    ntb = tok // tb
    nst = tb // P  # token subtiles per block
    fdim = NB * tb
    CB = NB * 2 * R  # const columns for B

    nc = bacc.Bacc("TRN2", target_bir_lowering=False, debug=False)
    xT = nc.dram_tensor("xT", [ntb, P, fdim], f16, kind="ExternalInput").ap()
    Bt = nc.dram_tensor("Bt", [P, NB, 2 * R], f16, kind="ExternalInput").ap()
    At = nc.dram_tensor("At", [2 * R, OUT], f16, kind="ExternalInput").ap()
    out = nc.dram_tensor("out", [tok, OUT], f16, kind="ExternalOutput").ap()

    with tile.TileContext(nc) as tc:
        with (
            tc.tile_pool(name="const", bufs=1) as const_pool,
            tc.tile_pool(name="xin", bufs=ntb) as x_pool,
            tc.tile_pool(name="part", bufs=2) as part_pool,
            tc.tile_pool(name="ps1", bufs=2, space="PSUM") as ps1,
            tc.tile_pool(name="ps2", bufs=3, space="PSUM") as ps2,
            tc.tile_pool(name="osb", bufs=4 * nst) as out_pool,
        ):
            # B ahead of the x blocks on the sync queue (mm1 needs it first);
            # A alone on the scalar queue, landing concurrently
            B_sb = const_pool.tile([P, NB, 2 * R], f16)
            nc.sync.dma_start(out=B_sb[:], in_=Bt[:])
            # A arrives as [A; zeros] (32 rows); two doubling copies build the
            # 4-row-group layout (rows 32g+r = A[r], rows 32g+16+.. = 0) with
            # no uninitialized SBUF in the matmul operand
            A_sb = const_pool.tile([P, OUT], f16)
            nc.scalar.dma_start(out=A_sb[0 : 2 * R, :], in_=At[:])
            for h in (32, 64):
                nc.vector.tensor_copy(A_sb[h : 2 * h, :], A_sb[0:h, :])

            def make_mm2(part_sb, o_sbs, tbi):
                """Emission closures for block tbi's mm2 + evacuation + stores."""
                emits = []
                for st in range(nst):
                    for op in range(OUT // 1024):

                        def emit(st=st, op=op, i=len(emits)):
                            ps_o = ps2.tile([P, 1024], f32)
                            for k in range(2):
                                nc.tensor.matmul(
                                    ps_o[:, k * 512 : (k + 1) * 512],
                                    lhsT=part_sb[:, st * P : (st + 1) * P],
                                    rhs=A_sb[:, (2 * op + k) * 512 : (2 * op + k + 1) * 512],
                                    start=True,
                                    stop=True,
                                    skip_group_check=True,
                                )
                            if i % 2 == 0:
                                nc.vector.tensor_copy(
                                    o_sbs[st][:, op * 1024 : (op + 1) * 1024], ps_o[:]
                                )
                            else:
                                nc.scalar.activation(
                                    o_sbs[st][:, op * 1024 : (op + 1) * 1024],
                                    ps_o[:],
                                    mybir.ActivationFunctionType.Copy,
                                )

                        emits.append(emit)

                def emit_stores():
                    for st in range(nst):
                        t0 = tbi * tb + st * P
                        if tbi == ntb - 1 and st == nst - 1:
                            # last tile: taper the store so the final DMA piece
                            # (whose completion latency ends the kernel) is small
                            for c0, c1 in ((0, 2048), (2048, 3072), (3072, 4096)):
                                nc.sync.dma_start(
                                    out=out[t0 : t0 + P, c0:c1],
                                    in_=o_sbs[st][:, c0:c1],
                                )
                        else:
                            nc.sync.dma_start(out=out[t0 : t0 + P, :], in_=o_sbs[st][:])

                return emits, emit_stores

            # dispatch every x load up-front: a store dispatch on the sync
            # engine is sem-gated on evacuation, and any load emitted after it
            # would be head-of-line blocked behind that wait
            x_tiles = []
            for tbi in range(ntb):
                xT_sb = x_pool.tile([P, fdim], f16)
                nc.sync.dma_start(out=xT_sb[:], in_=xT[tbi])
                x_tiles.append(xT_sb)

            pending, pending_stores = [], None
            for tbi in range(ntb):
                xT_sb = x_tiles[tbi]
                # drain half the previous block's mm2 before this block's mm1
                # (those only wait on PSUM banks, not on this block's x)
                nhead = len(pending) // 2
                for e in pending[:nhead]:
                    e()
                rest = pending[nhead:]
                # mm1: col group g accumulates chunks {4k+g} into partitions
                # [32g, 32g+32); rest of prev block's mm2 interleaves
                ps_part = ps1.tile([P, tb], f32)
                for c8 in range(NB // 4):
                    for g in range(4):
                        c = c8 * 4 + g
                        nc.tensor.matmul(
                            ps_part[32 * g : 32 * g + 2 * R, :],
                            lhsT=B_sb[:, c, :],
                            rhs=xT_sb[:, c * tb : (c + 1) * tb],
                            start=(c8 == 0),
                            stop=(c8 == NB // 4 - 1),
                            tile_position=(0, 32 * g),
                            skip_group_check=True,
                        )
                    for e in rest[c8 :: NB // 4]:
                        e()
                if pending_stores is not None:
                    pending_stores()
                part_sb = part_pool.tile([P, tb], f16, tag="part")
                nc.vector.tensor_copy(part_sb[:], ps_part[:])
                o_sbs = [
                    out_pool.tile([P, OUT], f16, name=f"osb{st}_{tbi}", tag=f"osb{st}")
                    for st in range(nst)
                ]
                pending, pending_stores = make_mm2(part_sb, o_sbs, tbi)
            for e in pending:
                e()
            pending_stores()

    nc.compile()
    _NC_CACHE[key] = nc
    return nc


TB = 256


def make_in_maps(x, lora_A, lora_B, n_cores=N_CORES):
    x = np.asarray(x, dtype=np.float32)
    A = np.asarray(lora_A, dtype=np.float32)
    B = np.asarray(lora_B, dtype=np.float32)
    xf = x.reshape(-1, IN)
    ntok = xf.shape[0] // n_cores
    tb = min(TB, ntok)
    # B chunk tiles, zero-padded to 32 wide (the zero columns make the mm1
    # partials zero in rows 32g+16.., which the mm2 contraction relies on)
    Bv = np.zeros((P, NB, 2 * R), dtype=np.float16)
    Bv[:, :, :R] = B.reshape(NB, P, R).transpose(1, 0, 2)
    A2 = np.zeros((2 * R, OUT), dtype=np.float16)
    A2[:R] = (A * np.float32(SCALE)).astype(np.float16)
    in_maps = []
    for c in range(n_cores):
        shard = xf[c * ntok : (c + 1) * ntok]
        # pre-tile: [ntb, 128, NB*tb]; xT[tbi,p,c*tb+t] = shard[tbi*tb+t, c*128+p]
        xt = np.ascontiguousarray(
            shard.reshape(ntok // tb, tb, NB, P)
            .transpose(0, 3, 2, 1)
            .reshape(ntok // tb, P, NB * tb),
            dtype=np.float16,
        )
        in_maps.append({"xT": xt, "Bt": Bv, "At": A2})
    return in_maps, ntok


def kernel_with_results(x, lora_A, lora_B, trace=False, **kwargs):
    from concourse.bass_utils import run_bass_kernel_spmd

    in_maps, ntok = make_in_maps(x, lora_A, lora_B)
    nc = build_nc(ntok, tb=TB)
    res = run_bass_kernel_spmd(nc, in_maps, list(range(N_CORES)), trace=trace, **kwargs)
    out = np.concatenate([r["out"] for r in res.results], axis=0).astype(np.float32)
    return out.reshape(np.asarray(x).shape[:-1] + (OUT,)), res


def kernel(x, lora_A, lora_B):
    out, _ = kernel_with_results(x, lora_A, lora_B)
    return out
